# revision 12
# baseline (speedup 1.0000x reference)
"""Trainium2 Bass kernel for nn_Attention_52012053955205.

Multi-head causal attention, B=2 S=2048 D=1024 H=16 HD=64, fp32.

Sharding: 8 cores = 2-way batch x 4-way heads. Each core computes, for its
batch item b and its 4 heads, the partial output sum_h z_h @ W_O_h  as a
full [S, D] tile; the host sums the 4 partials per batch and adds b_O.

Per-core dataflow (everything "transposed" so the softmax denominator is a
free by-product of matmuls):
  xT [D, S] (host-pretransposed) -> QT/KT [d_pair=128, S] via projection
  matmuls (W packed per head-pair, 1/sqrt(HD) folded into W_Q host-side).
  V [s, 4*64] natural layout; a ones column is appended per head
  (V' [s, 65]) so the z-matmul also produces the softmax denominator.
  Scores TRANSPOSED: S_T[k_tile, q] = KT_tile.T @ QT_block -- both heads
  of a pair as two K=64 matmuls into one 2-bank PSUM tile (disjoint PE
  row groups run concurrently). One [128, 1024] exp on ScalarE per k-tile
  (no max-subtraction: scores are bounded, exp fits fp32); causal masking
  via gpsimd affine_select (fill=0) on the Pool engine, only on the
  triangle-containing chunk of diagonal tiles.
  z_unnorm^T [65, q] accumulated over k tiles in PSUM (row 64 = denom).
  Normalization: reciprocal of denom row, broadcast to 128 partitions via
  K=1 matmuls against selector rows, one DVE multiply per head.
  Output projection: out[s, D] = znorm_pair^T.T @ W_O_pair, accumulated
  over the two head pairs in PSUM; bf16 partials DMA'd out and summed on
  the host in fp32.
  Bulk matmul operands are bf16 (same PE rate as fp32r here, half the
  DMA traffic); PSUM accumulation is fp32 and the softmax reciprocal
  stays tf32. End-to-end rel err ~3e-3 (gate 2e-2).

Scheduling: the ScalarE exp stream paces the attention inner loop, so the
PE work that does NOT depend on exps (QK/V projections of the next
s-block, out-projection of the previous q-block) is emitted as small
"filler" chunks BETWEEN the per-k-tile score/z matmuls. This keeps PE
busy during every exp. z matmuls are software-pipelined one k-tile
behind the exp that feeds them. PSUM budget: scores [128,2,512] double
buffered (8KB/part) + z accumulators (4KB) + a shared rotation for
out-proj / V-proj / broadcast tiles (4KB) = 16KB (all 8 banks).
"""

import json
from collections import deque

import numpy as np

B, S, D, H, HD = 2, 2048, 1024, 16, 64
NCORES = 8
HPC = 4  # heads per core

_STATE = {}


# ---------------------------------------------------------------------------
# Tile tail-drain workaround: walrus in this container rejects >2 sem waits
# on one instruction ("Too many sync wait commands"). Split the tail waits
# across one sync NOP per logical proc; the drain itself then needs none.
# ---------------------------------------------------------------------------
def _patch_tile_drain():
    import concourse.tile as tile
    from concourse.vector_clock import ScopedClock, VectorClock

    if getattr(tile.TileContext, "_drain_split_patch", False):
        return

    def _split_drain_and_barrier(self, tick_clock, wait_clock):
        gc = tick_clock.global_clock
        n = len(gc)
        for proc in range(n):
            t = gc[proc]
            if t > 0:
                vc = VectorClock([t if i == proc else 0 for i in range(n)])
                nop = self.nc.sync.nop(nofuse=True)
                wait_clock.add_sem_waits(nop.ins, ScopedClock({None: vc}))
        self.nc.sync.drain()
        self.nc.all_engine_barrier()
        assert self.sems is not None
        popped = self.nc._tile_sem_poison_stack.pop()
        assert popped is self._sem_poison
        self.nc.clear_and_free_semaphores(list(self.sems.allocated().values()))
        self.nc.all_engine_barrier()

    tile.TileContext._drain_and_barrier = _split_drain_and_barrier
    tile.TileContext._drain_split_patch = True


def _split_waits_bir(bir: bytes) -> bytes:
    """Walrus in this container allows only one sem wait per instruction.
    Spill extra on_wait entries onto same-engine NoOps inserted right
    before the instruction (the NX executes them in stream order)."""
    d = json.loads(bir)
    ctr = 0
    for f in d["functions"]:
        for bb in f["blocks"]:
            new = []
            for ins in bb["instructions"]:
                si = ins.get("sync_info")
                waits = si.get("on_wait", []) if si else []
                if len(waits) > 1:
                    for w in waits[:-1]:
                        ctr += 1
                        new.append(
                            {
                                "debug": ins.get("debug", 0),
                                "engine": ins["engine"],
                                "ins": [],
                                "name": f"I-wsplit-{ctr}",
                                "opcode": "NoOp",
                                "outs": [],
                                "sync_info": {"on_update": [], "on_wait": [w]},
                            }
                        )
                    si["on_wait"] = [waits[-1]]
                new.append(ins)
            bb["instructions"] = new
    return json.dumps(d).encode()


def _hook_wait_split(nc):
    orig = nc.to_json_bytes

    def patched():
        return _split_waits_bir(orig())

    nc.to_json_bytes = patched
    return nc


# ---------------------------------------------------------------------------
# Bass program (identical on all 8 cores; all per-core data arrives as
# ExternalInputs)
# ---------------------------------------------------------------------------
def _build_nc(reps=1, qk_bias=False, fill_budget=300, pool_mask=True, bf16=True,
              timing_mode=False, ablate=(), interleave=True):
    import concourse.bass as bass
    import concourse.mybir as mybir
    import concourse.tile as tile
    from concourse.alu_op_type import AluOpType

    FP = mybir.dt.float32
    FR = mybir.dt.float32r
    BF = mybir.dt.bfloat16
    AF = mybir.ActivationFunctionType
    _patch_tile_drain()

    nc = bass.Bass(target_bir_lowering=False)

    MT = BF if bf16 else FR  # dtype for bulk matmul operand tiles
    SR = FR  # selector/reciprocal dtype (keeps 1/denom at tf32 precision)

    mm = nc.tensor.matmul

    xT = nc.dram_tensor("xt", [D, S], MT, kind="ExternalInput")
    # weights arrive pre-arranged in their SBUF layouts so the loads are
    # fully contiguous per partition
    wq = nc.dram_tensor("wq", [128, 2, 8, 128], MT, kind="ExternalInput")
    wk = nc.dram_tensor("wk", [128, 2, 8, 128], MT, kind="ExternalInput")
    wv = nc.dram_tensor("wv", [128, 8, 256], MT, kind="ExternalInput")
    wo = nc.dram_tensor("wo", [128, 2, D], MT, kind="ExternalInput")
    if qk_bias:
        bq = nc.dram_tensor("bq", [2, 128], FP, kind="ExternalInput")
        bk = nc.dram_tensor("bk", [2, 128], FP, kind="ExternalInput")
    bv = nc.dram_tensor("bv", [256], FP, kind="ExternalInput")
    sel = nc.dram_tensor("sel", [2, 128], SR, kind="ExternalInput")
    OT = MT  # output dtype: bf16 partials, summed in fp32 on the host
    if timing_mode:
        # identical device-side work, but the bulk output stays in device
        # DRAM so chained timing calls don't round-trip the full output
        # over the PJRT tunnel; a 4-byte sentinel is the only external
        # output
        out = nc.dram_tensor("out", [S, D], OT, kind="Internal")
        tout = nc.dram_tensor("tout", [1, 4], FP, kind="ExternalOutput")
    else:
        out = nc.dram_tensor("out", [S, D], OT, kind="ExternalOutput")

    with tile.TileContext(nc) as tc:
        with (
            nc.allow_low_precision(reason="tf32 (fp32r) matmul pipeline"),
            tc.tile_pool(name="consts", bufs=1) as consts,
            tc.tile_pool(name="xp", bufs=3) as xp,
            tc.tile_pool(name="qk", bufs=1) as qk,
            tc.tile_pool(name="vp", bufs=1) as vp,
            tc.tile_pool(name="zp", bufs=1) as zp,
            tc.tile_pool(name="etp", bufs=4) as etp,
            tc.tile_pool(name="bcp", bufs=2) as bcp,
            tc.tile_pool(name="rdpool", bufs=4) as rdpool,
            tc.tile_pool(name="ostp", bufs=4) as ostp,
            tc.tile_pool(name="psS", bufs=2, space="PSUM") as psS_pool,
            tc.tile_pool(name="psZ", bufs=2, space="PSUM") as psZ_pool,
            tc.tile_pool(name="psO", bufs=2, space="PSUM") as psO_pool,
        ):
            # ---- constants ----
            # DMAs are serviced serially; order them so the first QK chains
            # and V chains unblock as early as possible.
            xTr = xT[:].rearrange("(c p) s -> p c s", p=128)
            wq_sb = consts.tile([128, 2, 8, 128], MT, tag="wq")
            # split per pair so the first projection chain only waits on
            # its own half
            nc.sync.dma_start(out=wq_sb[:, 0:1], in_=wq[:, 0:1])
            x_tiles = {}

            def emit_x(sb):
                x_t = xp.tile([128, 8, 512], MT, tag="x", name=f"x_{sb}")
                for dd in range(4):
                    nc.sync.dma_start(
                        out=x_t[:, 2 * dd : 2 * dd + 2, :],
                        in_=xTr[:, 2 * dd : 2 * dd + 2, sb * 512 : (sb + 1) * 512],
                    )
                x_tiles[sb] = x_t

            emit_x(0)
            wk_sb = consts.tile([128, 2, 8, 128], MT, tag="wk")
            nc.sync.dma_start(out=wk_sb[:, 0:1], in_=wk[:, 0:1])
            wv_sb = consts.tile([128, 8, 256], MT, tag="wv")
            nc.sync.dma_start(out=wv_sb, in_=wv[:])
            nc.sync.dma_start(out=wq_sb[:, 1:2], in_=wq[:, 1:2])
            nc.sync.dma_start(out=wk_sb[:, 1:2], in_=wk[:, 1:2])
            if qk_bias:
                bq_sb = consts.tile([128, 2], FP, tag="bq")
                nc.sync.dma_start(out=bq_sb, in_=bq[:].rearrange("a p -> p a"))
                bk_sb = consts.tile([128, 2], FP, tag="bk")
                nc.sync.dma_start(out=bk_sb, in_=bk[:].rearrange("a p -> p a"))
            bvbc_sb = consts.tile([128, 4, 64], FP, tag="bvbc")
            nc.sync.dma_start(
                out=bvbc_sb,
                in_=bass.AP(tensor=bv, offset=0, ap=[[0, 128], [1, 256]]),
            )
            # group B: not needed until normalization / out-proj
            wo_sb = consts.tile([128, 2, D], MT, tag="wo")
            sel_sb = consts.tile([1, 2, 128], SR, tag="sel")

            def emit_const_group_b():
                nc.sync.dma_start(
                    out=sel_sb,
                    in_=bass.AP(
                        tensor=sel, offset=0, ap=[[256, 1], [128, 2], [1, 128]]
                    ),
                )
                nc.sync.dma_start(out=wo_sb, in_=wo[:])

            # ---------------- filler machinery ----------------
            # Two queues of (est_pe_ns, closure) chunks pulled into the
            # attention inner loop to keep PE busy during exps:
            #  - deadline_q: projection chains tagged with the (segment,
            #    kt) by which they MUST be emitted (due-monotone FIFO);
            #    drained lazily at their due point, pulled eagerly before.
            #  - fillers: no-deadline work (out-proj), pulled when the
            #    deadline queue is empty, drained at rep end.
            deadline_q = deque()  # (cost, fn, due=(seg, kt), mark)
            fillers = deque()  # (cost, fn, mark)
            _last_ost = {}

            _cur_due = [None]
            # chains that hold PSUM pool slots across several items must be
            # fully emitted before anything else allocates from their pool
            # (else the PE stream deadlocks on the slot semaphore). mark is
            # +1 at a slot-acquiring item, -1 at the item that emits the
            # last reader of those slots.
            _open = [0]

            def add_filler(cost, fn, mark=0):
                if _cur_due[0] is not None:
                    deadline_q.append((cost, fn, _cur_due[0], mark))
                else:
                    fillers.append((cost, fn, mark))

            def _emit(fn, mark):
                _open[0] += mark
                fn()

            def drain_due(seg, kt):
                while deadline_q and deadline_q[0][2] <= (seg, kt):
                    _, fn, _, mk = deadline_q.popleft()
                    _emit(fn, mk)

            def pull_fillers(budget):
                while budget > 0 and (deadline_q or fillers):
                    if deadline_q:
                        c, fn, _, mk = deadline_q.popleft()
                    else:
                        c, fn, mk = fillers.popleft()
                    _emit(fn, mk)
                    budget -= c

            def close_open_chains():
                while _open[0] > 0 and (deadline_q or fillers):
                    if deadline_q:
                        _, fn, _, mk = deadline_q.popleft()
                    else:
                        _, fn, mk = fillers.popleft()
                    _emit(fn, mk)

            def drain_fillers():
                while deadline_q:
                    _, fn, _, mk = deadline_q.popleft()
                    _emit(fn, mk)
                while fillers:
                    _, fn, mk = fillers.popleft()
                    _emit(fn, mk)

            # ---------------- emitters ----------------
            # Q and K chains are interleaved MM-by-MM (and V chains pairwise)
            # so consecutive PE matmuls never accumulate into the same PSUM
            # bank back-to-back.
            def _qk_chain(sb, pair, qt_sb, kt_sb):
                """Q+K projection chains for one pair, bank-interleaved."""
                x_t = x_tiles[sb]
                ps = {}

                def chunk(c):
                    if c == 0:
                        ps["q"] = psO_pool.tile(
                            [128, 512], FP, tag="O", name=f"psQ_{sb}_{pair}"
                        )
                        ps["k"] = psO_pool.tile(
                            [128, 512], FP, tag="O", name=f"psK_{sb}_{pair}"
                        )
                    for w_sb, key in ((wq_sb, "q"), (wk_sb, "k")):
                        mm(
                            ps[key],
                            w_sb[:, pair, c, :],
                            x_t[:, c, :],
                            start=(c == 0),
                            stop=(c == 7),
                        )

                def fin(which):
                    dst = (qt_sb if which == 0 else kt_sb)[
                        :, pair, sb * 512 : (sb + 1) * 512
                    ]
                    src = ps["q" if which == 0 else "k"]
                    if qk_bias:
                        nc.scalar.activation(
                            dst, src, AF.Identity,
                            bias=(bq_sb if which == 0 else bk_sb)[
                                :, pair : pair + 1
                            ],
                        )
                    else:
                        nc.vector.tensor_copy(dst, src)

                add_filler(430, lambda: chunk(0), mark=1)
                for c in range(1, 8):
                    add_filler(430, lambda c=c: chunk(c))
                add_filler(100, lambda: fin(0))
                add_filler(100, lambda: fin(1), mark=-1)

            def _qk_chain_old(sb, pair, which, w_sb, qt_sb, kt_sb):
                x_t = x_tiles[sb]
                ps = {}

                def first():
                    ps["t"] = psO_pool.tile(
                        [128, 512], FP, tag="O", name=f"psQ_{sb}_{pair}_{which}"
                    )
                    chunk(0)

                def chunk(c0):
                    p = ps["t"]
                    for c in range(c0, c0 + 2):
                        mm(
                            p,
                            w_sb[:, pair, c, :],
                            x_t[:, c, :],
                            start=(c == 0),
                            stop=(c == 7),
                        )

                def fin():
                    dst = (qt_sb if which == 0 else kt_sb)[
                        :, pair, sb * 512 : (sb + 1) * 512
                    ]
                    if qk_bias:
                        nc.scalar.activation(
                            dst, ps["t"], AF.Identity,
                            bias=(bq_sb if which == 0 else bk_sb)[
                                :, pair : pair + 1
                            ],
                        )
                    else:
                        nc.vector.tensor_copy(dst, ps["t"])

                add_filler(430, first, mark=1)
                for c0 in (2, 4, 6):
                    add_filler(430, lambda c0=c0, chunk=chunk: chunk(c0))
                add_filler(100, fin, mark=-1)

            def _v_chain(sb, stl0, v_sb):
                """V projection chains for s-tiles stl0/stl0+1, interleaved."""
                x_t = x_tiles[sb]
                ps = {}

                def chunk2(c0):
                    if c0 == 0:
                        for i in range(2):
                            ps[i] = psO_pool.tile(
                                [128, 256], FP, tag="O",
                                name=f"psV_{sb * 4 + stl0 + i}",
                            )
                    for c in (c0, c0 + 1):
                        for i in range(2):
                            stl = stl0 + i
                            mm(
                                ps[i],
                                x_t[:, c, stl * 128 : (stl + 1) * 128],
                                wv_sb[:, c, :],
                                start=(c == 0),
                                stop=(c == 7),
                            )

                def fin(i):
                    nc.vector.tensor_add(
                        v_sb[:, sb * 4 + stl0 + i, :, 0:64],
                        ps[i].rearrange("p (h d) -> p h d", h=4),
                        bvbc_sb,
                    )

                add_filler(430, lambda: chunk2(0), mark=1)
                for c0 in (2, 4, 6):
                    add_filler(430, lambda c0=c0: chunk2(c0))
                add_filler(100, lambda: fin(0))
                add_filler(100, lambda: fin(1), mark=-1)

            def _v_chain_old(sb, stl, v_sb):
                x_t = x_tiles[sb]
                st = sb * 4 + stl
                ps = {}

                def first():
                    ps["t"] = psO_pool.tile(
                        [128, 256], FP, tag="O", name=f"psV_{st}"
                    )
                    chunk(0)

                def chunk(c0):
                    p = ps["t"]
                    for c in range(c0, c0 + 4):
                        mm(
                            p,
                            x_t[:, c, stl * 128 : (stl + 1) * 128],
                            wv_sb[:, c, :],
                            start=(c == 0),
                            stop=(c == 7),
                        )

                def fin():
                    nc.vector.tensor_add(
                        v_sb[:, st, :, 0:64],
                        ps["t"].rearrange("p (h d) -> p h d", h=4),
                        bvbc_sb,
                    )

                add_filler(430, first, mark=1)
                add_filler(430, lambda chunk=chunk: chunk(4))
                add_filler(100, fin, mark=-1)

            def add_v_fillers(sb, v_sb):
                if interleave:
                    for stl0 in (0, 2):
                        _v_chain(sb, stl0, v_sb)
                else:
                    for stl in range(4):
                        _v_chain_old(sb, stl, v_sb)

            def _out_st(Qb, st, znp, alt_pool=False):
                soff = (st % 4) * 128
                ps = {}
                # tail out-proj can borrow the (idle) score PSUM slots so
                # consecutive s-tiles pipeline across 4 slots instead of 2
                pool, tag = (
                    (psS_pool, "S") if alt_pool and st % 2 else (psO_pool, "O")
                )

                def mms(pair):
                    # both D-halves per item, so consecutive MMs alternate
                    # PSUM banks (and consecutive MMs share one lhsT)
                    if pair == 0:
                        ps["ost"] = ostp.tile(
                            [128, D], OT, tag="ost", name=f"ost_{st}"
                        )
                        _last_ost["t"] = ps["ost"]
                        for Db in range(2):
                            ps[Db] = pool.tile(
                                [128, 512], FP, tag=tag, name=f"psO_{st}_{Db}"
                            )
                    for Db in range(2):
                        mm(
                            ps[Db],
                            znp[:, pair, Qb, soff : soff + 128],
                            wo_sb[:, pair, Db * 512 : (Db + 1) * 512],
                            start=(pair == 0),
                            stop=(pair == 1),
                        )

                def mms_old(Db):
                    p = pool.tile([128, 512], FP, tag=tag, name=f"psO_{st}_{Db}")
                    if Db == 0:
                        ps["ost"] = ostp.tile(
                            [128, D], OT, tag="ost", name=f"ost_{st}"
                        )
                        _last_ost["t"] = ps["ost"]
                    ps[Db] = p
                    for pair in range(2):
                        mm(
                            p,
                            znp[:, pair, Qb, soff : soff + 128],
                            wo_sb[:, pair, Db * 512 : (Db + 1) * 512],
                            start=(pair == 0),
                            stop=(pair == 1),
                        )

                def copy_store(Db):
                    # copy + store each 512-wide half separately so the
                    # first half streams out while the second is computed.
                    # In the tail (alt_pool) ScalarE is idle, so it takes
                    # half the copies off the DVE-bound critical path.
                    dst = ps["ost"][:, Db * 512 : (Db + 1) * 512]
                    if alt_pool and Db == 1:
                        nc.scalar.activation(dst, ps[Db], AF.Copy)
                    else:
                        nc.vector.tensor_copy(dst, ps[Db])
                    # odd halves issue from the (idle) Pool sequencer so
                    # the tail stores overlap instead of serializing on
                    # the SP DMA stream
                    eng = nc.gpsimd if (alt_pool and Db == 1) else nc.sync
                    eng.dma_start(
                        out=out[
                            st * 128 : (st + 1) * 128, Db * 512 : (Db + 1) * 512
                        ],
                        in_=dst,
                    )

                if interleave:
                    add_filler(430, lambda mms=mms: mms(0), mark=1)
                    add_filler(430, lambda mms=mms: mms(1))
                    add_filler(120, lambda f=copy_store: f(0))
                    add_filler(120, lambda f=copy_store: f(1), mark=-1)
                else:
                    add_filler(430, lambda f=mms_old: f(0), mark=1)
                    add_filler(120, lambda f=copy_store: f(0))
                    add_filler(430, lambda f=mms_old: f(1))
                    add_filler(120, lambda f=copy_store: f(1), mark=-1)

            def add_out_fillers(Qb, znp, half=None, alt_pool=False):
                """Output projection + store for the 4 s-tiles of q-block
                Qb (requires normalized znp for Qb)."""
                sts = range(4 * Qb, 4 * Qb + 4)
                if half == 0:
                    sts = sts[:2]
                elif half == 1:
                    sts = sts[2:]
                for st in sts:
                    _out_st(Qb, st, znp, alt_pool=alt_pool)

            def emit_attn(pair, Qb, qt_sb, kt_sb, v_sb, znp):
                """Attention for one head pair and one 512-wide q-block,
                z pipelined one k-tile behind, fillers interleaved."""
                seg = 2 * Qb + pair
                q0, q1 = Qb * 512, (Qb + 1) * 512
                ktmax = 4 * (Qb + 1)
                psZs = []
                for hh in range(2):
                    psZ_h = psZ_pool.tile(
                        [65, 512], FP, tag="Z", name=f"psZ_{pair}_{Qb}_{hh}"
                    )
                    psZs.append(psZ_h)
                    if "z" in ablate:
                        nc.vector.memset(psZ_h[:, 0:1], 1.0)

                def emit_z(kt, e_t, r):
                    if "z" in ablate:
                        return
                    for hh in range(2):
                        mm(
                            psZs[hh][:, r:512],
                            v_sb[:, kt, 2 * pair + hh, :],
                            e_t[:, hh, r:512],
                            start=(kt == 0),
                            stop=(kt == ktmax - 1),
                        )

                pending = None  # (kt, e_t, r) -- z emitted one kt behind
                for kt in range(ktmax):
                    # emit projection chains whose deadline has arrived
                    drain_due(seg, kt)
                    # diagonal k-tiles: q-columns < r are fully masked, so
                    # scores/exp/z are all computed on [r:512] only
                    diag = kt >= 4 * Qb
                    j = kt - 4 * Qb
                    r = j * 128 if diag else 0
                    if not bf16:
                        # fp32r needs N>=256 for full rate
                        r = min(r, 256)
                    psS = None
                    if "score" not in ablate:
                        psS = psS_pool.tile(
                            [128, 2, 512], FP, tag="S", name=f"psS_{pair}_{Qb}_{kt}"
                        )
                        for hh in range(2):
                            po = hh * 64
                            mm(
                                psS[:, hh, r:512],
                                kt_sb[po : po + 64, pair, kt * 128 : (kt + 1) * 128],
                                qt_sb[po : po + 64, pair, q0 + r : q1],
                                start=True,
                                stop=True,
                            )
                    e_t = etp.tile(
                        [128, 2, 512], MT, tag="et", name=f"et_{pair}_{Qb}_{kt}"
                    )
                    if "exp" not in ablate:
                        src = (
                            psS[:, :, r:512]
                            if psS is not None
                            else qt_sb[:, :, q0 + r : q1]
                        )
                        nc.scalar.activation(e_t[:, :, r:512], src, AF.Exp)
                    elif psS is not None:
                        nc.vector.tensor_copy(
                            e_t[:, :, r : r + 1], psS[:, :, r : r + 1]
                        )
                    else:
                        nc.vector.memset(e_t[:, :, r : r + 1], 1.0)
                    if diag:
                        # causal mask on the Pool engine: zero where
                        # q_global < k_global, only on the chunk(s) that
                        # contain masked elements: cols [r, (j+1)*128)
                        c0, c1 = r, (j + 1) * 128
                        for hh in range(2):
                            if pool_mask:
                                nc.gpsimd.affine_select(
                                    e_t[:, hh, c0:c1],
                                    e_t[:, hh, c0:c1],
                                    pattern=[[1, c1 - c0]],
                                    compare_op=AluOpType.is_ge,
                                    fill=0.0,
                                    base=Qb * 512 + c0 - kt * 128,
                                    channel_multiplier=-1,
                                )
                            else:
                                nc.vector.tensor_mul(
                                    e_t[:, hh, c0:c1],
                                    e_t[:, hh, c0:c1],
                                    masks_sb[:, j, c0 - r : c1 - r],
                                )
                    # budget: exp duration (incl fixed overhead) minus the
                    # attn PE work of this kt (z + next S + sem slack);
                    # fill_budget=300 is the calibrated neutral point. The
                    # first two k-tiles get extra budget to cover the
                    # previous pair's normalization latency (psZ slots),
                    # and fillers are emitted BEFORE the pipelined z so
                    # they run while that latency resolves.
                    exp_tot = int(2 * (512 - r) * 0.833) + 190
                    budget = exp_tot - 840 + (fill_budget - 300)
                    if kt < 2:
                        budget += 1200
                    if budget > 0:
                        pull_fillers(budget)
                    if pending is not None:
                        emit_z(*pending)
                    pending = (kt, e_t, r)
                emit_z(*pending)

                # ---- normalization ----
                # DVE: recip0, recip1, mul0, mul1; ACT: zc shift-copy +
                # bcs copy run in parallel with the recips, shortening the
                # serial chain that holds the psZ slots. The recips are
                # emitted BEFORE the filler pull so pulled fillers' DVE
                # epilogues queue behind them, not ahead.
                rds = []
                for hh in range(2):
                    rd_h = rdpool.tile(
                        [1, 512], SR, tag="rd", name=f"rd_{pair}_{Qb}_{hh}"
                    )
                    rds.append(rd_h)
                    nc.vector.reciprocal(rd_h, psZs[hh][64:65, :])
                # cover the reciprocal latency before the bc matmul
                pull_fillers(860)
                close_open_chains()
                zc = bcp.tile([128, 512], FP, tag="zc", name=f"zc_{pair}_{Qb}")
                nc.scalar.activation(zc[64:128, :], psZs[1][0:64, :], AF.Copy)
                bc = psO_pool.tile([128, 512], FP, tag="O", name=f"bc_{pair}_{Qb}")
                mm(bc, sel_sb[:, 0, :], rds[0], start=True, stop=False)
                mm(bc, sel_sb[:, 1, :], rds[1], start=False, stop=True)
                bcs = bcp.tile([128, 512], FP, tag="bcs", name=f"bcs_{pair}_{Qb}")
                nc.scalar.activation(bcs, bc, AF.Copy)
                nc.vector.tensor_mul(
                    znp[0:64, pair, Qb, :],
                    psZs[0][0:64, :],
                    bcs[0:64, :],
                )
                nc.vector.tensor_mul(
                    znp[64:128, pair, Qb, :],
                    zc[64:128, :],
                    bcs[64:128, :],
                )

            masks_sb = None
            if not pool_mask:
                masks = nc.dram_tensor("masks", [4, 128, 512], MT,
                                       kind="ExternalInput")

            for _rep in range(reps):
                qt_sb = qk.tile([128, 2, S], MT, tag="qt")
                kt_sb = qk.tile([128, 2, S], MT, tag="kt")
                v_sb = vp.tile([128, 16, 4, 65], MT, tag="v")
                znp = zp.tile([128, 2, 4, 512], MT, tag="zn")
                if not pool_mask and masks_sb is None:
                    masks_sb = consts.tile([128, 4, 512], MT, tag="masks")
                    nc.sync.dma_start(
                        out=masks_sb, in_=masks[:].rearrange("m p j -> p m j")
                    )
                # ones column of V' (written once; proj fills the rest)
                if bf16:
                    nc.vector.memset(v_sb[:, :, :, 64:65], 1.0)
                else:
                    nc.vector.memset(v_sb[:, :, :, 64:65].bitcast(FP), 1.0)

                if _rep == 0:
                    emit_const_group_b()

                # Deadlines (seg = 2*sb + pair): Q(sb, pair) is read from
                # kt=0 of attn(pair, sb); K(sb, pair) and V(sb) only from
                # kt=4*sb on. Chains are queued due-monotone and pulled
                # eagerly as stall filler before their due point; out-proj
                # work has no deadline and fills whatever remains.
                def queue_block(sb):
                    if interleave:
                        _cur_due[0] = (2 * sb, 0)
                        _qk_chain(sb, 0, qt_sb, kt_sb)
                        _cur_due[0] = (2 * sb, 4 * sb)
                        add_v_fillers(sb, v_sb)
                        _cur_due[0] = (2 * sb + 1, 0)
                        _qk_chain(sb, 1, qt_sb, kt_sb)
                        _cur_due[0] = None
                        return
                    _cur_due[0] = (2 * sb, 0)
                    _qk_chain_old(sb, 0, 0, wq_sb, qt_sb, kt_sb)
                    _cur_due[0] = (2 * sb, 4 * sb)
                    _qk_chain_old(sb, 0, 1, wk_sb, qt_sb, kt_sb)
                    add_v_fillers(sb, v_sb)
                    _cur_due[0] = (2 * sb + 1, 0)
                    _qk_chain_old(sb, 1, 0, wq_sb, qt_sb, kt_sb)
                    _cur_due[0] = (2 * sb + 1, 4 * sb)
                    _qk_chain_old(sb, 1, 1, wk_sb, qt_sb, kt_sb)
                    _cur_due[0] = None

                queue_block(0)
                for sb in range(4):
                    if sb + 1 <= 3:
                        emit_x(sb + 1)
                        queue_block(sb + 1)
                    # out(1) is deferred from sb=2 (whose pulls are busy
                    # with the block-3 projection deadlines) to sb=3,
                    # which otherwise starves for tail filler
                    if sb == 1:
                        add_out_fillers(0, znp)
                    elif sb == 3:
                        add_out_fillers(1, znp)
                        add_out_fillers(2, znp)
                    emit_attn(0, sb, qt_sb, kt_sb, v_sb, znp)
                    emit_attn(1, sb, qt_sb, kt_sb, v_sb, znp)
                # tail: out-proj of the last q-block
                add_out_fillers(3, znp, alt_pool=True)
                drain_fillers()
            if timing_mode:
                # sentinel depends on real results so the computation
                # cannot be dead-code-eliminated
                tsb = consts.tile([1, 4], FP, tag="tout")
                nc.vector.tensor_copy(tsb, _last_ost["t"][0:1, 0:4])
                nc.sync.dma_start(out=tout[:], in_=tsb)

    return _hook_wait_split(nc)


# ---------------------------------------------------------------------------
# Persistent PJRT runner (mirrors run_bass_via_pjrt, but keeps the jitted
# callable so repeated kernel() calls don't recompile)
# ---------------------------------------------------------------------------
class _Runner:
    def __init__(self, nc):
        import jax
        import jax.numpy as jnp  # noqa: F401
        import numpy as _np
        from jax.experimental.shard_map import shard_map
        from jax.sharding import Mesh, PartitionSpec
        import concourse.mybir as mybir
        from concourse.bass2jax import (
            _bass_exec_p,
            install_neuronx_cc_hook,
            partition_id_tensor,
        )

        install_neuronx_cc_hook()
        self.jax = jax
        pname = nc.partition_id_tensor.name if nc.partition_id_tensor else None
        in_names, out_names, out_avals, zero_outs = [], [], [], []
        for alloc in nc.m.functions[0].allocations:
            if not isinstance(alloc, mybir.MemoryLocationSet):
                continue
            name = alloc.memorylocations[0].name
            if alloc.kind == "ExternalInput":
                if name == pname:
                    continue
                in_names.append(name)
            elif alloc.kind == "ExternalOutput":
                shape = tuple(alloc.tensor_shape)
                dtype = mybir.dt.np(alloc.dtype)
                out_names.append(name)
                out_avals.append(jax.core.ShapedArray(shape, dtype))
                zero_outs.append(_np.zeros(shape, dtype))
        self.in_names, self.out_names = list(in_names), list(out_names)
        self.out_avals, self.zero_outs = out_avals, zero_outs
        n_params, n_outs = len(in_names), len(out_names)
        self.n_params = n_params
        all_names = in_names + out_names
        if pname is not None:
            all_names = all_names + [pname]

        def _body(*args):
            operands = list(args)
            if pname is not None:
                operands.append(partition_id_tensor())
            outs = _bass_exec_p.bind(
                *operands,
                out_avals=tuple(out_avals),
                in_names=tuple(all_names),
                out_names=tuple(out_names),
                lowering_input_output_aliases=(),
                sim_require_finite=True,
                sim_require_nnan=True,
                nc=nc,
            )
            return tuple(outs)

        devices = jax.devices()[:NCORES]
        mesh = Mesh(np.asarray(devices), ("core",))
        in_specs = (PartitionSpec("core"),) * (n_params + n_outs)
        out_specs = (PartitionSpec("core"),) * n_outs
        self.fn = jax.jit(
            shard_map(
                _body,
                mesh=mesh,
                in_specs=in_specs,
                out_specs=out_specs,
                check_rep=False,
            ),
            donate_argnums=tuple(range(n_params, n_params + n_outs)),
            keep_unused=True,
        )

    def device_put_inputs(self, concat_in):
        return [self.jax.device_put(a) for a in concat_in]

    def time_exec(self, dev_in, iters=8):
        """Repeat execution with device-resident inputs; the previous call's
        (donated, fully-overwritten) outputs serve as the next call's output
        buffers, so nothing moves over the axon tunnel."""
        import time as _time

        zeros = [
            np.zeros((NCORES * z.shape[0], *z.shape[1:]), z.dtype)
            for z in self.zero_outs
        ]
        r = self.fn(*dev_in, *zeros)
        self.jax.block_until_ready(r)
        times = []
        for _ in range(iters):
            t0 = _time.perf_counter()
            r = self.fn(*dev_in, *r)
            self.jax.block_until_ready(r)
            times.append(_time.perf_counter() - t0)
        return times

    def concat_inputs(self, in_maps):
        return [
            np.concatenate([in_maps[c][n] for c in range(NCORES)], axis=0)
            for n in self.in_names
        ]

    def run_concat(self, concat_in):
        zeros = [
            np.zeros((NCORES * z.shape[0], *z.shape[1:]), z.dtype)
            for z in self.zero_outs
        ]
        outs = self.fn(*concat_in, *zeros)
        outs = [np.asarray(o) for o in outs]
        return outs

    def run(self, in_maps):
        outs = self.run_concat(self.concat_inputs(in_maps))
        per_core = []
        for c in range(NCORES):
            m = {}
            for i, n in enumerate(self.out_names):
                shp = self.out_avals[i].shape
                m[n] = outs[i].reshape(NCORES, *shp)[c]
            per_core.append(m)
        return per_core


def _round_tf32(a):
    """Round fp32 -> TF32 (10-bit mantissa, RNE) so device-side fp32r
    consumers see pre-rounded values."""
    u = np.ascontiguousarray(a, dtype=np.float32).view(np.uint32)
    r = (u + np.uint32(0x1000) + ((u >> np.uint32(13)) & np.uint32(1))) & np.uint32(0xFFFFE000)
    return r.view(np.float32)


def _make_masks():
    """0/1 multiplicative causal masks for the 4 diagonal k-tile offsets."""
    m = np.ones((4, 128, 512), dtype=np.float32)
    for r in range(4):
        p = np.arange(128)[:, None]
        j = np.arange(512)[None, :]
        m[r][p + 128 * r > j] = 0.0
    return _round_tf32(m)


def _prep_core_inputs(inputs, bf16=True):
    """Shard + repack the full problem inputs into per-core input maps."""
    if bf16:
        import ml_dtypes

        cast = lambda a: np.ascontiguousarray(a, dtype=np.float32).astype(  # noqa: E731
            ml_dtypes.bfloat16
        )
    else:
        cast = _round_tf32
    x = np.asarray(inputs["normalized_resid_pre"], dtype=np.float32)
    W_Q = np.asarray(inputs["W_Q"], dtype=np.float32)
    W_K = np.asarray(inputs["W_K"], dtype=np.float32)
    W_V = np.asarray(inputs["W_V"], dtype=np.float32)
    W_O = np.asarray(inputs["W_O"], dtype=np.float32)
    b_Q = np.asarray(inputs["b_Q"], dtype=np.float32)
    b_K = np.asarray(inputs["b_K"], dtype=np.float32)
    b_V = np.asarray(inputs["b_V"], dtype=np.float32)

    scale = np.float32(1.0 / np.sqrt(HD))
    masks = _make_masks()
    sel = np.zeros((2, 128), dtype=np.float32)
    sel[0, 0:64] = 1.0
    sel[1, 64:128] = 1.0

    in_maps = []
    for c in range(NCORES):
        b, g = c // 4, c % 4
        hs = [4 * g + i for i in range(HPC)]
        xTb = cast(np.ascontiguousarray(x[b].T))  # [D, S]
        wq_p = np.zeros((2, D, 128), dtype=np.float32)
        wk_p = np.zeros((2, D, 128), dtype=np.float32)
        wo_p = np.zeros((2, 128, D), dtype=np.float32)
        bq_p = np.zeros((2, 128), dtype=np.float32)
        bk_p = np.zeros((2, 128), dtype=np.float32)
        for pr in range(2):
            h0, h1 = hs[2 * pr], hs[2 * pr + 1]
            wq_p[pr, :, 0:64] = W_Q[h0] * scale
            wq_p[pr, :, 64:128] = W_Q[h1] * scale
            wk_p[pr, :, 0:64] = W_K[h0]
            wk_p[pr, :, 64:128] = W_K[h1]
            wo_p[pr, 0:64, :] = W_O[h0]
            wo_p[pr, 64:128, :] = W_O[h1]
            bq_p[pr, 0:64] = b_Q[h0] * scale
            bq_p[pr, 64:128] = b_Q[h1] * scale
            bk_p[pr, 0:64] = b_K[h0]
            bk_p[pr, 64:128] = b_K[h1]
        wv_p = np.concatenate([W_V[h] for h in hs], axis=1)  # [D, 256]
        # pre-arrange into SBUF layouts: partition dim first
        wq_p = wq_p.reshape(2, 8, 128, 128).transpose(2, 0, 1, 3)
        wk_p = wk_p.reshape(2, 8, 128, 128).transpose(2, 0, 1, 3)
        wv_p = wv_p.reshape(8, 128, 256).transpose(1, 0, 2)
        wo_p = wo_p.transpose(1, 0, 2)  # [128, 2, D]
        wq_p, wk_p, wv_p, wo_p = (
            cast(wq_p),
            cast(wk_p),
            cast(wv_p),
            cast(wo_p),
        )
        bv_p = np.concatenate([b_V[h] for h in hs], axis=0)  # [256]
        in_maps.append(
            {
                "xt": xTb,
                "wq": wq_p,
                "wk": wk_p,
                "wv": np.ascontiguousarray(wv_p),
                "wo": wo_p,
                "bq": bq_p,
                "bk": bk_p,
                "bv": np.ascontiguousarray(bv_p),
                "masks": masks,
                "sel": sel,
            }
        )
    return in_maps


def _get_state(qk_bias=False):
    if qk_bias not in _STATE:
        _STATE[qk_bias] = _Runner(_build_nc(qk_bias=qk_bias))
    return _STATE[qk_bias]


def kernel(**inputs):
    need_qk_bias = bool(
        np.any(np.asarray(inputs["b_Q"])) or np.any(np.asarray(inputs["b_K"]))
    )
    st = _get_state(qk_bias=need_qk_bias)
    in_maps = _prep_core_inputs(inputs)
    per_core = st.run(in_maps)
    b_O = np.asarray(inputs["b_O"], dtype=np.float32)
    out = np.zeros((B, S, D), dtype=np.float32)
    for c in range(NCORES):
        out[c // 4] += per_core[c]["out"].astype(np.float32)
    out += b_O[None, None, :]
    return out

